# revision 30
# baseline (speedup 1.0000x reference)
"""CapsLayer2D Trainium2 kernel (8-core SPMD, data-parallel over batch).

Math per position p (of B*R*C) and capsule n:
  U[n,i,o] = sum_e x[p,i,e] * W[n,i,e,o]          (u_hat)
  b0 = 1/64; 2x { v = squash(sum_i b*U); b += sum_o U*v }; out = squash(sum_i b*U)

Routing is algebraically restated without the b-state:
  s_mean = (1/64) sum_i U_i ; v0 = squash(s_mean); s(0) = s_mean
  iter t: a_i = U_i . v_t ; s(t+1) = s(t) + sum_i a_i U_i ; v_{t+1} = squash(s(t+1))
(identical results: b_t = 1/64 + U.(v0+..+v_{t-1}) telescopes into s).

Mapping:
  - 8 cores, 2 batches each -> 392 positions/core, 4 pos-blocks of 98.
  - Per block: s_mean via one dense bf16 matmul (K=1024, N=160); u_hat via
    block-diagonal-W bf16 matmuls (PSUM cols (gi,o,n2)), ACT-drained into
    U[p, (gi, o, n)] bf16; then 2 routing iterations on DVE.
    All routing DVE ops run in 2x mode: broadcasts sit on non-innermost dims,
    tree-sum halves stay contiguous (o middle -> o-tree 2x; gi outermost ->
    i-tree halves are contiguous monoliths).
  - Output v-layout is (o, n) per position; host transposes to (n, o).
"""
import numpy as np

import concourse.bacc as bacc
import concourse.bass as bass
import concourse.mybir as mybir
import concourse.tile as tile
from concourse.bass_utils import run_bass_kernel_spmd

N_CORES = 8
B, R, C = 16, 14, 14
N_IN, D_IN = 64, 16          # i, e
N_CAPS, CAPS_DIM = 10, 16    # n, o
IE = N_IN * D_IN             # 1024
POS = (B // N_CORES) * R * C # 392 positions per core
BLK = 98                     # pos-block size
NBLK = POS // BLK            # 4
NF = N_CAPS // 2             # 5 units of 2 capsules
NCH = IE // 128              # 8 contraction chunks
F32 = mybir.dt.float32
BF16 = mybir.dt.bfloat16
AF = mybir.ActivationFunctionType


def _squash_on(nc, pool, s_ap, v_ap):
    """v = squash(s) in (o, n) free layout. s_ap [P,160] f32, v_ap [P,160].

    Square and the Sqrt-independent steps run on DVE before the single ACT
    Sqrt dependency, minimizing the DVE stall on the ACT round-trip."""
    P = s_ap.shape[0]
    sq = pool.tile([P, 160], F32, tag="sq")
    nc.vector.tensor_mul(sq[:], s_ap, s_ap)
    q = pool.tile([P, N_CAPS], F32, tag="q")
    nc.vector.tensor_reduce(q[:], sq[:].rearrange("p (o n) -> p n o", n=N_CAPS),
                            axis=mybir.AxisListType.X, op=mybir.AluOpType.add)
    qp = pool.tile([P, N_CAPS], F32, tag="qp")
    nc.vector.tensor_scalar_add(qp[:], q[:], 1.0)
    rc = pool.tile([P, N_CAPS], F32, tag="rc")
    nc.vector.reciprocal(rc[:], qp[:])
    rt = pool.tile([P, N_CAPS], F32, tag="rt")
    nc.scalar.activation(rt[:], q[:], AF.Sqrt)
    al = pool.tile([P, N_CAPS], F32, tag="al")
    nc.vector.tensor_mul(al[:], rt[:], rc[:])
    alb = al[:].unsqueeze(1).broadcast_to([P, CAPS_DIM, N_CAPS])
    nc.vector.tensor_mul(v_ap.rearrange("p (o n) -> p o n", n=N_CAPS),
                         s_ap.rearrange("p (o n) -> p o n", n=N_CAPS), alb)


def build_kernel(dbg=False, repeat=1):
    nc = bacc.Bacc("TRN2", target_bir_lowering=False, debug=False,
                   num_devices=N_CORES)
    xTb = nc.dram_tensor("xTb", [IE, POS], BF16, kind="ExternalInput").ap()
    # bdw: [128=(i8,e), (g,f) * 256=(i8,o,n2)] block-diag W, bf16
    bdw = nc.dram_tensor("bdw", [128, NCH * NF * 256], BF16,
                         kind="ExternalInput").ap()
    # wdb: [IE, 160=(o,n)] dense W for s_mean
    wdb = nc.dram_tensor("wdb", [IE, N_CAPS * 16], BF16,
                         kind="ExternalInput").ap()
    # out rows = positions, cols = (o, n)
    out = nc.dram_tensor("out", [POS, N_CAPS * 16], F32,
                         kind="ExternalOutput").ap()

    with tile.TileContext(nc) as tc:
        for _rep in range(repeat):
            with tc.tile_pool(name="const", bufs=1) as const, \
                 tc.tile_pool(name="work", bufs=2) as work, \
                 tc.tile_pool(name="ubp", bufs=2) as ubp, \
                 tc.tile_pool(name="big", bufs=1) as big, \
                 tc.tile_pool(name="psum_u", bufs=2, space="PSUM") as psum_u:
                # Warm the ACT function tables (Copy/Sqrt) during the input
                # DMAs instead of paying the ~1.3us load on the critical path.
                warm = const.tile([1, 2], F32)
                nc.vector.memset(warm[:], 1.0)
                nc.scalar.activation(warm[:, 0:1], warm[:, 1:2], AF.Copy)
                nc.scalar.activation(warm[:, 0:1], warm[:, 1:2], AF.Sqrt)
                # xtb + wd first: they gate s_mean(0); bdw only gates u_hat(0).
                # Spread issue across engine queues so HWDGE issue pipelines.
                dmae = [nc.sync, nc.scalar, nc.sync, nc.scalar]
                xtb_t = const.tile([128, NCH * POS], BF16)
                wd_t = const.tile([128, NCH * 160], BF16)
                bdw_t = const.tile([128, NCH * NF * 256], BF16)
                for g in range(NCH):   # interleave so u_hat(0) streams early
                    nc.sync.dma_start(xtb_t[:, g * POS:(g + 1) * POS],
                                      xTb[g * 128:(g + 1) * 128, :])
                    nc.scalar.dma_start(
                        bdw_t[:, g * NF * 256:(g + 1) * NF * 256],
                        bdw[:, g * NF * 256:(g + 1) * NF * 256])
                for g in range(NCH):
                    nc.gpsimd.dma_start(wd_t[:, g * 160:(g + 1) * 160],
                                        wdb[g * 128:(g + 1) * 128, :])
                sacc = const.tile([BLK, NBLK * 160], F32)   # s per block, (o,n)
                v0_t = const.tile([BLK, NBLK * 160], BF16)

                # ---- prologue: s_mean(b) for all blocks; v0 = squash ----
                # (keeps these short ACT/DVE chains off the routing's critical
                # path -- the U-drains otherwise queue ahead of them on ACT)
                for b in range(NBLK):
                    sb = sacc[:, b * 160:(b + 1) * 160]
                    ps = psum_u.tile([BLK, 160], F32, tag="ps")
                    for g in range(NCH):
                        nc.tensor.matmul(
                            ps[:],
                            xtb_t[:, g * POS + b * BLK: g * POS + (b + 1) * BLK],
                            wd_t[:, g * 160:(g + 1) * 160],
                            start=(g == 0), stop=(g == NCH - 1))
                    nc.scalar.activation(sb, ps[:], AF.Copy, scale=1.0 / N_IN)
                # batched squash over all 4 blocks: one Sqrt round-trip
                sq4 = work.tile([BLK, NBLK * 160], F32, tag="sq4")
                nc.vector.tensor_mul(sq4[:], sacc[:], sacc[:])
                q4t = work.tile([BLK, NBLK * N_CAPS], F32, tag="q4t")
                nc.vector.tensor_reduce(
                    q4t[:],
                    sq4[:].rearrange("p (b o n) -> p b n o", b=NBLK, n=N_CAPS),
                    axis=mybir.AxisListType.X, op=mybir.AluOpType.add)
                qp4 = work.tile([BLK, NBLK * N_CAPS], F32, tag="qp4")
                nc.vector.tensor_scalar_add(qp4[:], q4t[:], 1.0)
                rc4 = work.tile([BLK, NBLK * N_CAPS], F32, tag="rc4")
                nc.vector.reciprocal(rc4[:], qp4[:])
                rt4 = work.tile([BLK, NBLK * N_CAPS], F32, tag="rt4")
                nc.scalar.activation(rt4[:], q4t[:], AF.Sqrt)
                al4 = work.tile([BLK, NBLK * N_CAPS], F32, tag="al4")
                nc.vector.tensor_mul(al4[:], rt4[:], rc4[:])
                al4b = al4[:].rearrange("p (b n) -> p b n", b=NBLK) \
                    .unsqueeze(2).broadcast_to([BLK, NBLK, CAPS_DIM, N_CAPS])
                nc.vector.tensor_mul(
                    v0_t[:].rearrange("p (b o n) -> p b o n", b=NBLK, n=N_CAPS),
                    sacc[:].rearrange("p (b o n) -> p b o n", b=NBLK, n=N_CAPS),
                    al4b)

                for b in range(NBLK):
                    sb = sacc[:, b * 160:(b + 1) * 160]
                    v0 = v0_t[:, b * 160:(b + 1) * 160]
                    # ---- u_hat(b) -> U[p, (gi, o, n)] bf16 ----
                    U = ubp.tile([BLK, 10240], BF16, tag="U")
                    Uv = U[:].rearrange("p (gi o n) -> p gi o n",
                                        gi=64, o=16, n=N_CAPS)
                    for h in range(2):   # chunk halves: g in [4h, 4h+4)
                        for f in range(NF):  # h-major: early chunks drain first
                            up = psum_u.tile([BLK, 1024], F32, tag="up")
                            for gg in range(4):
                                g = 4 * h + gg
                                nc.tensor.matmul(
                                    up[:, gg * 256:(gg + 1) * 256],
                                    xtb_t[:, g * POS + b * BLK: g * POS + (b + 1) * BLK],
                                    bdw_t[:, (g * NF + f) * 256:(g * NF + f + 1) * 256],
                                    start=True, stop=True)
                            # PSUM cols (i8,o,n2) per chunk -> merged (gi,o,n2)
                            nc.scalar.activation(
                                Uv[:, 32 * h:32 * (h + 1), :, 2 * f:2 * f + 2],
                                up[:].rearrange("p (gi o n) -> p gi o n",
                                                gi=32, o=16, n=2),
                                AF.Copy)

                    # ---- 2 routing iterations ----
                    for it in range(2):
                        v_ap = v0 if it == 0 else v1[:]
                        # P = U * v (bcast over gi: middle dims stay 2x)
                        P = big.tile([BLK, 10240], BF16, tag="P")
                        Pv4 = P[:].rearrange("p (gi o n) -> p gi o n",
                                             gi=64, o=16, n=N_CAPS)
                        if b == 0 and it == 0:
                            # pipeline fill: split per (f,h)-slice so the mul
                            # starts as soon as each U-drain lands
                            for h in range(2):
                                for f in range(NF):
                                    sl = (slice(None), slice(32 * h, 32 * h + 32),
                                          slice(None), slice(2 * f, 2 * f + 2))
                                    vbs = v_ap.rearrange("p (o n) -> p o n",
                                                         n=N_CAPS) \
                                        [:, :, 2 * f:2 * f + 2].unsqueeze(1) \
                                        .broadcast_to([BLK, 32, 16, 2])
                                    nc.vector.tensor_mul(Pv4[sl], Uv[sl], vbs)
                        else:
                            vb = v_ap.rearrange("p (o n) -> p o n", n=N_CAPS) \
                                .unsqueeze(1).broadcast_to([BLK, 64, 16, N_CAPS])
                            nc.vector.tensor_mul(Pv4, Uv, vb)
                        # o-tree (middle-dim halves, contiguous runs)
                        with nc.allow_low_precision("bf16 tree sums"):
                            Pv = P[:].rearrange("p (gi o n) -> p gi o n",
                                                gi=64, o=16, n=N_CAPS)
                            t1 = big.tile([BLK, 5120], BF16, tag="t1")
                            t1v = t1[:].rearrange("p (gi o n) -> p gi o n",
                                                  gi=64, o=8, n=N_CAPS)
                            nc.vector.tensor_add(t1v, Pv[:, :, 0:8, :],
                                                 Pv[:, :, 8:16, :])
                            t2 = big.tile([BLK, 2560], BF16, tag="t2")
                            t2v = t2[:].rearrange("p (gi o n) -> p gi o n",
                                                  gi=64, o=4, n=N_CAPS)
                            nc.vector.tensor_add(t2v, t1v[:, :, 0:4, :],
                                                 t1v[:, :, 4:8, :])
                            t3 = big.tile([BLK, 1280], BF16, tag="t3")
                            t3v = t3[:].rearrange("p (gi o n) -> p gi o n",
                                                  gi=64, o=2, n=N_CAPS)
                            nc.vector.tensor_add(t3v, t2v[:, :, 0:2, :],
                                                 t2v[:, :, 2:4, :])
                            a = big.tile([BLK, 640], BF16, tag="a")
                            av = a[:].rearrange("p (gi o n) -> p gi o n",
                                                gi=64, o=1, n=N_CAPS)
                            nc.vector.tensor_add(av, t3v[:, :, 0:1, :],
                                                 t3v[:, :, 1:2, :])
                        # Q = U * a (bcast over o: middle dim, still 2x)
                        Q = big.tile([BLK, 10240], BF16, tag="Q")
                        ab = a[:].rearrange("p (gi n) -> p gi n", n=N_CAPS) \
                            .unsqueeze(2).broadcast_to([BLK, 64, 16, N_CAPS])
                        nc.vector.tensor_mul(
                            Q[:].rearrange("p (gi o n) -> p gi o n",
                                           gi=64, o=16, n=N_CAPS), Uv, ab)
                        # i-tree (outermost gi halves: contiguous monoliths)
                        with nc.allow_low_precision("bf16 tree sums"):
                            q1 = big.tile([BLK, 5120], BF16, tag="q1")
                            nc.vector.tensor_add(q1[:], Q[:, 0:5120],
                                                 Q[:, 5120:10240])
                            q2 = big.tile([BLK, 2560], BF16, tag="q2")
                            nc.vector.tensor_add(q2[:], q1[:, 0:2560],
                                                 q1[:, 2560:5120])
                            q3 = big.tile([BLK, 1280], BF16, tag="q3")
                            nc.vector.tensor_add(q3[:], q2[:, 0:1280],
                                                 q2[:, 1280:2560])
                            q4 = big.tile([BLK, 640], BF16, tag="q4")
                            nc.vector.tensor_add(q4[:], q3[:, 0:640],
                                                 q3[:, 640:1280])
                            q5 = big.tile([BLK, 320], BF16, tag="q5")
                            nc.vector.tensor_add(q5[:], q4[:, 0:320],
                                                 q4[:, 320:640])
                            inc = work.tile([BLK, 160], F32, tag="inc")
                            nc.vector.tensor_add(inc[:], q5[:, 0:160],
                                                 q5[:, 160:320])
                        nc.vector.tensor_add(sb, sb, inc[:])
                        if it == 0:
                            v1 = work.tile([BLK, 160], BF16, tag="v1")
                            _squash_on(nc, work, sb, v1[:])
                        else:
                            out_t = work.tile([BLK, 160], F32, tag="out_t")
                            _squash_on(nc, work, sb, out_t[:])
                            nc.sync.dma_start(
                                out[b * BLK:(b + 1) * BLK, :], out_t[:])
    nc.compile()
    return nc


def _host_prep(inputs, W):
    """Build per-core input maps from full inputs."""
    import ml_dtypes
    x = np.ascontiguousarray(inputs, dtype=np.float32).reshape(B, R * C, IE)
    Wf = np.ascontiguousarray(W, dtype=np.float32)  # [n, i, e, o]
    # bdw[(i8_r,e), (g, f, i8, o, n2)]: delta(i8_r,i8) * W[2f+n2, 8g+i8, e, o]
    Wg = Wf.reshape(NF, 2, NCH, 8, D_IN, CAPS_DIM)  # [f, n2, g, i8, e, o]
    bdw7 = np.zeros((8, D_IN, NCH, NF, 8, CAPS_DIM, 2), dtype=np.float32)
    for i8 in range(8):
        # [f, n2, g, e, o] -> [e, g, f, o, n2]
        bdw7[i8, :, :, :, i8, :, :] = Wg[:, :, :, i8, :, :].transpose(3, 2, 0, 4, 1)
    bdw = bdw7.reshape(128, NCH * NF * 256).astype(ml_dtypes.bfloat16)
    # wdb[(i,e), (o,n)]
    wdb = np.ascontiguousarray(
        Wf.transpose(1, 2, 3, 0).reshape(IE, CAPS_DIM * N_CAPS)
    ).astype(ml_dtypes.bfloat16)
    bpc = B // N_CORES
    in_maps = []
    for c in range(N_CORES):
        xc = x[c * bpc:(c + 1) * bpc].reshape(POS, IE)
        in_maps.append({
            "xTb": np.ascontiguousarray(xc.T).astype(ml_dtypes.bfloat16),
            "bdw": bdw,
            "wdb": wdb,
        })
    return in_maps


_NC_CACHE = []


def kernel(inputs: np.ndarray, W: np.ndarray) -> np.ndarray:
    in_maps = _host_prep(inputs, W)
    if not _NC_CACHE:
        _NC_CACHE.append(build_kernel())
    nc = _NC_CACHE[0]
    res = run_bass_kernel_spmd(nc, in_maps, list(range(N_CORES)))
    outs = [res.results[c]["out"] for c in range(N_CORES)]
    full = np.concatenate(outs, axis=0)  # [3136, (o,n)]
    return np.ascontiguousarray(
        full.reshape(B, R, C, CAPS_DIM, N_CAPS).transpose(0, 1, 2, 4, 3))


# revision 49
# speedup vs baseline: 2.9680x; 2.9680x over previous
"""CapsLayer2D Trainium2 kernel (8-core SPMD, data-parallel over batch).

Math per position p (of B*R*C) and capsule n:
  U[n,i,o] = sum_e x[p,i,e] * W[n,i,e,o]          (u_hat)
  b0 = 1/64; 2x { v = squash(sum_i b*U); b += sum_o U*v }; out = squash(sum_i b*U)

Routing is algebraically restated without the b-state:
  s_mean = (1/64) sum_i U_i ; v0 = squash(s_mean); s(0) = s_mean
  iter t: a_i = U_i . v_t ; s(t+1) = s(t) + sum_i a_i U_i ; v_{t+1} = squash(s(t+1))
(identical results: b_t = 1/64 + U.(v0+..+v_{t-1}) telescopes into s).

Mapping:
  - 8 cores, 2 batches each -> 392 positions/core, 4 pos-blocks of 98.
  - Per block: s_mean via one dense bf16 matmul (K=1024, N=160); u_hat via
    block-diagonal-W bf16 matmuls (PSUM cols (gi,o,n2)), ACT-drained into
    U[p, (gi, o, n)] bf16; then 2 routing iterations on DVE.
    All routing DVE ops run in 2x mode: broadcasts sit on non-innermost dims,
    tree-sum halves stay contiguous (o middle -> o-tree 2x; gi outermost ->
    i-tree halves are contiguous monoliths).
  - Output v-layout is (o, n) per position; host transposes to (n, o).
"""
import numpy as np

import concourse.bacc as bacc
import concourse.bass as bass
import concourse.mybir as mybir
import concourse.tile as tile
from concourse.bass_utils import run_bass_kernel_spmd

N_CORES = 8
B, R, C = 16, 14, 14
N_IN, D_IN = 64, 16          # i, e
N_CAPS, CAPS_DIM = 10, 16    # n, o
IE = N_IN * D_IN             # 1024
POS = (B // N_CORES) * R * C # 392 positions per core
BLK = 98                     # pos-block size
NBLK = POS // BLK            # 4
NF = N_CAPS // 2             # 5 units of 2 capsules
NCH = IE // 128              # 8 contraction chunks
F32 = mybir.dt.float32
BF16 = mybir.dt.bfloat16
AF = mybir.ActivationFunctionType


def _squash_on(nc, pool, s_ap, v_ap):
    """v = squash(s) in (o, n) free layout. s_ap [P,160] f32, v_ap [P,160].

    Square and the Sqrt-independent steps run on DVE before the single ACT
    Sqrt dependency, minimizing the DVE stall on the ACT round-trip."""
    P = s_ap.shape[0]
    sq = pool.tile([P, 160], F32, tag="sq")
    nc.vector.tensor_mul(sq[:], s_ap, s_ap)
    q = pool.tile([P, N_CAPS], F32, tag="q")
    nc.vector.tensor_reduce(q[:], sq[:].rearrange("p (o n) -> p n o", n=N_CAPS),
                            axis=mybir.AxisListType.X, op=mybir.AluOpType.add)
    qp = pool.tile([P, N_CAPS], F32, tag="qp")
    nc.vector.tensor_scalar_add(qp[:], q[:], 1.0)
    rc = pool.tile([P, N_CAPS], F32, tag="rc")
    nc.vector.reciprocal(rc[:], qp[:])
    rt = pool.tile([P, N_CAPS], F32, tag="rt")
    nc.scalar.activation(rt[:], q[:], AF.Sqrt)
    al = pool.tile([P, N_CAPS], F32, tag="al")
    nc.vector.tensor_mul(al[:], rt[:], rc[:])
    alb = al[:].unsqueeze(1).broadcast_to([P, CAPS_DIM, N_CAPS])
    nc.vector.tensor_mul(v_ap.rearrange("p (o n) -> p o n", n=N_CAPS),
                         s_ap.rearrange("p (o n) -> p o n", n=N_CAPS), alb)


def build_kernel(dbg=False, repeat=1):
    nc = bacc.Bacc("TRN2", target_bir_lowering=False, debug=False,
                   num_devices=N_CORES)
    xTb = nc.dram_tensor("xTb", [IE, POS], BF16, kind="ExternalInput").ap()
    # bdw: [128=(i8,e), (g,f) * 256=(i8,o,n2)] block-diag W, bf16
    bdw = nc.dram_tensor("bdw", [128, NCH * NF * 256], BF16,
                         kind="ExternalInput").ap()
    # wdb: [IE, 160=(o,n)] dense W for s_mean
    wdb = nc.dram_tensor("wdb", [IE, N_CAPS * 16], BF16,
                         kind="ExternalInput").ap()
    # out rows = positions, cols = (o, n)
    out = nc.dram_tensor("out", [POS, N_CAPS * 16], F32,
                         kind="ExternalOutput").ap()

    with tile.TileContext(nc) as tc:
        for _rep in range(repeat):
            with tc.tile_pool(name="const", bufs=1) as const, \
                 tc.tile_pool(name="work", bufs=2) as work, \
                 tc.tile_pool(name="ubp", bufs=2) as ubp, \
                 tc.tile_pool(name="big", bufs=1) as big, \
                 tc.tile_pool(name="psum_u", bufs=2, space="PSUM") as psum_u:
                # Warm the ACT function tables (Copy/Sqrt) during the input
                # DMAs instead of paying the ~1.3us load on the critical path.
                warm = const.tile([1, 2], F32)
                nc.vector.memset(warm[:], 1.0)
                nc.scalar.activation(warm[:, 0:1], warm[:, 1:2], AF.Copy)
                nc.scalar.activation(warm[:, 0:1], warm[:, 1:2], AF.Sqrt)
                # xtb + wd first: they gate s_mean(0); bdw only gates u_hat(0).
                # Spread issue across engine queues so HWDGE issue pipelines.
                dmae = [nc.sync, nc.scalar, nc.sync, nc.scalar]
                xtb_t = const.tile([128, NCH * POS], BF16)
                wd_t = const.tile([128, NCH * 160], BF16)
                bdw_t = const.tile([128, NCH * NF * 256], BF16)
                for g in range(NCH):   # interleave so u_hat(0) streams early
                    nc.sync.dma_start(xtb_t[:, g * POS:(g + 1) * POS],
                                      xTb[g * 128:(g + 1) * 128, :])
                    nc.scalar.dma_start(
                        bdw_t[:, g * NF * 256:(g + 1) * NF * 256],
                        bdw[:, g * NF * 256:(g + 1) * NF * 256])
                for g in range(NCH):
                    nc.gpsimd.dma_start(wd_t[:, g * 160:(g + 1) * 160],
                                        wdb[g * 128:(g + 1) * 128, :])
                sacc = const.tile([BLK, NBLK * 160], F32)   # s per block, (o,n)
                v0_t = const.tile([BLK, NBLK * 160], BF16)

                # ---- prologue: s_mean(b) for all blocks; v0 = squash ----
                # (keeps these short ACT/DVE chains off the routing's critical
                # path -- the U-drains otherwise queue ahead of them on ACT)
                for b in range(NBLK):
                    sb = sacc[:, b * 160:(b + 1) * 160]
                    ps = psum_u.tile([BLK, 160], F32, tag="ps")
                    for g in range(NCH):
                        nc.tensor.matmul(
                            ps[:],
                            xtb_t[:, g * POS + b * BLK: g * POS + (b + 1) * BLK],
                            wd_t[:, g * 160:(g + 1) * 160],
                            start=(g == 0), stop=(g == NCH - 1))
                    nc.scalar.activation(sb, ps[:], AF.Copy, scale=1.0 / N_IN)
                # batched squash over all 4 blocks: one Sqrt round-trip
                sq4 = work.tile([BLK, NBLK * 160], F32, tag="sq4")
                nc.vector.tensor_mul(sq4[:], sacc[:], sacc[:])
                q4t = work.tile([BLK, NBLK * N_CAPS], F32, tag="q4t")
                nc.vector.tensor_reduce(
                    q4t[:],
                    sq4[:].rearrange("p (b o n) -> p b n o", b=NBLK, n=N_CAPS),
                    axis=mybir.AxisListType.X, op=mybir.AluOpType.add)
                qp4 = work.tile([BLK, NBLK * N_CAPS], F32, tag="qp4")
                nc.vector.tensor_scalar_add(qp4[:], q4t[:], 1.0)
                rc4 = work.tile([BLK, NBLK * N_CAPS], F32, tag="rc4")
                nc.vector.reciprocal(rc4[:], qp4[:])
                rt4 = work.tile([BLK, NBLK * N_CAPS], F32, tag="rt4")
                nc.scalar.activation(rt4[:], q4t[:], AF.Sqrt)
                al4 = work.tile([BLK, NBLK * N_CAPS], F32, tag="al4")
                nc.vector.tensor_mul(al4[:], rt4[:], rc4[:])
                al4b = al4[:].rearrange("p (b n) -> p b n", b=NBLK) \
                    .unsqueeze(2).broadcast_to([BLK, NBLK, CAPS_DIM, N_CAPS])
                nc.vector.tensor_mul(
                    v0_t[:].rearrange("p (b o n) -> p b o n", b=NBLK, n=N_CAPS),
                    sacc[:].rearrange("p (b o n) -> p b o n", b=NBLK, n=N_CAPS),
                    al4b)

                for b in range(NBLK):
                    sb = sacc[:, b * 160:(b + 1) * 160]
                    v0 = v0_t[:, b * 160:(b + 1) * 160]
                    # ---- u_hat(b) -> U[p, (gi, o, n)] bf16 ----
                    U = ubp.tile([BLK, 10240], BF16, tag="U")
                    Uv = U[:].rearrange("p (gi o n) -> p gi o n",
                                        gi=64, o=16, n=N_CAPS)
                    for h in range(2):   # chunk halves: g in [4h, 4h+4)
                        for f in range(NF):  # h-major: early chunks drain first
                            up = psum_u.tile([BLK, 1024], F32, tag="up")
                            for gg in range(4):
                                g = 4 * h + gg
                                nc.tensor.matmul(
                                    up[:, gg * 256:(gg + 1) * 256],
                                    xtb_t[:, g * POS + b * BLK: g * POS + (b + 1) * BLK],
                                    bdw_t[:, (g * NF + f) * 256:(g * NF + f + 1) * 256],
                                    start=True, stop=True)
                            # PSUM cols (i8,o,n2) per chunk -> merged (gi,o,n2)
                            nc.scalar.activation(
                                Uv[:, 32 * h:32 * (h + 1), :, 2 * f:2 * f + 2],
                                up[:].rearrange("p (gi o n) -> p gi o n",
                                                gi=32, o=16, n=2),
                                AF.Copy)

                    # ---- 2 routing iterations ----
                    for it in range(2):
                        v_ap = v0 if it == 0 else v1[:]
                        # P = U * v (bcast over gi: middle dims stay 2x)
                        P = big.tile([BLK, 10240], BF16, tag="P")
                        Pv4 = P[:].rearrange("p (gi o n) -> p gi o n",
                                             gi=64, o=16, n=N_CAPS)
                        if b == 0 and it == 0:
                            # pipeline fill: split per (f,h)-slice so the mul
                            # starts as soon as each U-drain lands
                            for h in range(2):
                                for f in range(NF):
                                    sl = (slice(None), slice(32 * h, 32 * h + 32),
                                          slice(None), slice(2 * f, 2 * f + 2))
                                    vbs = v_ap.rearrange("p (o n) -> p o n",
                                                         n=N_CAPS) \
                                        [:, :, 2 * f:2 * f + 2].unsqueeze(1) \
                                        .broadcast_to([BLK, 32, 16, 2])
                                    nc.vector.tensor_mul(Pv4[sl], Uv[sl], vbs)
                        else:
                            vb = v_ap.rearrange("p (o n) -> p o n", n=N_CAPS) \
                                .unsqueeze(1).broadcast_to([BLK, 64, 16, N_CAPS])
                            nc.vector.tensor_mul(Pv4, Uv, vb)
                        # o-tree (middle-dim halves, contiguous runs)
                        with nc.allow_low_precision("bf16 tree sums"):
                            Pv = P[:].rearrange("p (gi o n) -> p gi o n",
                                                gi=64, o=16, n=N_CAPS)
                            t1 = big.tile([BLK, 5120], BF16, tag="t1")
                            t1v = t1[:].rearrange("p (gi o n) -> p gi o n",
                                                  gi=64, o=8, n=N_CAPS)
                            nc.vector.tensor_add(t1v, Pv[:, :, 0:8, :],
                                                 Pv[:, :, 8:16, :])
                            t2 = big.tile([BLK, 2560], BF16, tag="t2")
                            t2v = t2[:].rearrange("p (gi o n) -> p gi o n",
                                                  gi=64, o=4, n=N_CAPS)
                            nc.vector.tensor_add(t2v, t1v[:, :, 0:4, :],
                                                 t1v[:, :, 4:8, :])
                            t3 = big.tile([BLK, 1280], BF16, tag="t3")
                            t3v = t3[:].rearrange("p (gi o n) -> p gi o n",
                                                  gi=64, o=2, n=N_CAPS)
                            nc.vector.tensor_add(t3v, t2v[:, :, 0:2, :],
                                                 t2v[:, :, 2:4, :])
                            a = big.tile([BLK, 640], BF16, tag="a")
                            av = a[:].rearrange("p (gi o n) -> p gi o n",
                                                gi=64, o=1, n=N_CAPS)
                            nc.vector.tensor_add(av, t3v[:, :, 0:1, :],
                                                 t3v[:, :, 1:2, :])
                        # Q = U * a (bcast over o: middle dim, still 2x)
                        Q = big.tile([BLK, 10240], BF16, tag="Q")
                        ab = a[:].rearrange("p (gi n) -> p gi n", n=N_CAPS) \
                            .unsqueeze(2).broadcast_to([BLK, 64, 16, N_CAPS])
                        nc.vector.tensor_mul(
                            Q[:].rearrange("p (gi o n) -> p gi o n",
                                           gi=64, o=16, n=N_CAPS), Uv, ab)
                        # i-tree (outermost gi halves: contiguous monoliths)
                        with nc.allow_low_precision("bf16 tree sums"):
                            q1 = big.tile([BLK, 5120], BF16, tag="q1")
                            nc.vector.tensor_add(q1[:], Q[:, 0:5120],
                                                 Q[:, 5120:10240])
                            q2 = big.tile([BLK, 2560], BF16, tag="q2")
                            nc.vector.tensor_add(q2[:], q1[:, 0:2560],
                                                 q1[:, 2560:5120])
                            q3 = big.tile([BLK, 1280], BF16, tag="q3")
                            nc.vector.tensor_add(q3[:], q2[:, 0:1280],
                                                 q2[:, 1280:2560])
                            q4 = big.tile([BLK, 640], BF16, tag="q4")
                            nc.vector.tensor_add(q4[:], q3[:, 0:640],
                                                 q3[:, 640:1280])
                            q5 = big.tile([BLK, 320], BF16, tag="q5")
                            nc.vector.tensor_add(q5[:], q4[:, 0:320],
                                                 q4[:, 320:640])
                            inc = work.tile([BLK, 160], F32, tag="inc")
                            nc.vector.tensor_add(inc[:], q5[:, 0:160],
                                                 q5[:, 160:320])
                        nc.vector.tensor_add(sb, sb, inc[:])
                        if it == 0:
                            v1 = work.tile([BLK, 160], BF16, tag="v1")
                            _squash_on(nc, work, sb, v1[:])
                        else:
                            out_t = work.tile([BLK, 160], F32, tag="out_t")
                            _squash_on(nc, work, sb, out_t[:])
                            nc.sync.dma_start(
                                out[b * BLK:(b + 1) * BLK, :], out_t[:])
    nc.compile()
    return nc


def _host_prep(inputs, W):
    """Build per-core input maps from full inputs."""
    import ml_dtypes
    x = np.ascontiguousarray(inputs, dtype=np.float32).reshape(B, R * C, IE)
    Wf = np.ascontiguousarray(W, dtype=np.float32)  # [n, i, e, o]
    # bdw[(i8_r,e), (g, f, i8, o, n2)]: delta(i8_r,i8) * W[2f+n2, 8g+i8, e, o]
    Wg = Wf.reshape(NF, 2, NCH, 8, D_IN, CAPS_DIM)  # [f, n2, g, i8, e, o]
    bdw7 = np.zeros((8, D_IN, NCH, NF, 8, CAPS_DIM, 2), dtype=np.float32)
    for i8 in range(8):
        # [f, n2, g, e, o] -> [e, g, f, o, n2]
        bdw7[i8, :, :, :, i8, :, :] = Wg[:, :, :, i8, :, :].transpose(3, 2, 0, 4, 1)
    bdw = bdw7.reshape(128, NCH * NF * 256).astype(ml_dtypes.bfloat16)
    # wdb[(i,e), (o,n)]
    wdb = np.ascontiguousarray(
        Wf.transpose(1, 2, 3, 0).reshape(IE, CAPS_DIM * N_CAPS)
    ).astype(ml_dtypes.bfloat16)
    bpc = B // N_CORES
    in_maps = []
    for c in range(N_CORES):
        xc = x[c * bpc:(c + 1) * bpc].reshape(POS, IE)
        in_maps.append({
            "xTb": np.ascontiguousarray(xc.T).astype(ml_dtypes.bfloat16),
            "bdw": bdw,
            "wdb": wdb,
        })
    return in_maps


_NC_CACHE = []


def kernel(inputs: np.ndarray, W: np.ndarray) -> np.ndarray:
    in_maps = _host_prep(inputs, W)
    if not _NC_CACHE:
        _NC_CACHE.append(build_kernel())
    nc = _NC_CACHE[0]
    res = run_bass_kernel_spmd(nc, in_maps, list(range(N_CORES)))
    outs = [res.results[c]["out"] for c in range(N_CORES)]
    full = np.concatenate(outs, axis=0)  # [3136, (o,n)]
    return np.ascontiguousarray(
        full.reshape(B, R, C, CAPS_DIM, N_CAPS).transpose(0, 1, 2, 4, 3))
